# revision 57
# baseline (speedup 1.0000x reference)
"""Trainium2 Bass kernel for nn_DependencyParser.

Math (reference):
  pm = [zeros(1,H); sentence]                         # [1025, 300]
  s1 = sentence @ W1[:, :H].T                          # [1024, 100]
  s2 = pm @ W1[:, H:].T                                # [1025, 100]
  hidden = sigmoid(s1[i,m] + s2[j,m] + b1[m])          # [1024, 1025, 100]
  scores = hidden @ W2.T + b2                          # [1024, 1025]
  loss = mean |scores - onehot(target)|
  out = softmax(scores, axis=1)

Sharding: rows i split across 8 cores (128 rows each); weights replicated.

Per-core layout ("packed" scheme): each ACT instruction evaluates sigmoid for
32 i's x 4 m's using all 128 partitions: partition p = 32*m_l + i_l covers
(i = 32*i_g + i_l, m = 4*m_g + m_l).  The input tile R holds s2b rows
[4*m_g .. 4*m_g+4) each replicated 32x — built by a broadcast DMA from a DRAM
staging copy of s2b (for m_g in MG_PE, by a K=4 pattern matmul into PSUM so
the first sigmoids skip the DRAM roundtrip).  The per-partition ACT bias
carries s1[i, m] (precomputed into B via 48 small matmuls).  The W2
contraction is one M=128 matmul per (m_g, i_g) with a sparse block weight
matrix (a sliding 128-column window of w2blk), PSUM-accumulated over all 100
(m_g, i_g) steps into three bank-sized score chunks.  Matmuls use float32r
(TF32-like, full rate, ~2^-11 rounding) so the sigmoid outputs stay in fp32.

The sigmoid itself is computed as 0.5 + 0.5*tanh(z/2) with the affine folded
into host-side constants (W1/b1 halved, W2 halved, b2 shifted by 0.5*sum(W2))
— tanh shares the ACT table set with exp, so the softmax needs no table-set
switch.  For DRAM-fed m-groups the per-i-group s1 bias is added by the (idle)
VectorEngine into a 4-block-wide z tile, so ONE tanh instruction covers all
four i-groups ([128 x 4104]) — amortizing the ~224-cycle per-instruction ACT
overhead 4x (ACT busy ~94us instead of ~107us).  Rows are padded to 1026 so
the DVE bias-adds stay in their 2x perf mode.

The ACT engine is the bottleneck by design; PE (~56us), DVE (~60us) and DMA
(~60us) all hide underneath.  Cost-model total ~120.6us/core: ~13us pipeline
fill + ~95us tanh span + ~9us softmax/loss tail.
"""

import sys

if "/opt/trn_rl_repo" not in sys.path:
    sys.path.insert(0, "/opt/trn_rl_repo")

import numpy as np
import ml_dtypes

N = 1024
H = 300
MID = 100
NP1 = 1025
NCORES = 8
NLOC = N // NCORES  # 128 rows per core
NCHUNKS = [(0, 512), (512, 1024), (1024, 1025)]  # PSUM-bank-sized N chunks
KCH_A = [(0, 128), (128, 256), (256, 300)]  # K chunks for s1 (300)
KCH_B = [(0, 128), (128, 256), (256, 301)]  # K chunks for s2 (300 + bias row)

USE_F32R = True  # float32r (TF32-like) matmuls: full-rate + better precision than bf16
_PROGRAM = None


def _build_program():
    import concourse.bass as bass
    import concourse.tile as tile
    from concourse import bacc, mybir

    f32 = mybir.dt.float32
    f32r = mybir.dt.float32r
    bf16 = mybir.dt.bfloat16
    fr = f32r if USE_F32R else f32

    nc = bacc.Bacc(trn_type="TRN2")

    # ---- I/O ----
    sentT = nc.dram_tensor("sentT", [H, NLOC], f32, kind="ExternalInput")
    w1aTr = nc.dram_tensor("w1aTr", [H, MID], f32, kind="ExternalInput")
    w1bTx = nc.dram_tensor("w1bTx", [H + 1, MID], fr, kind="ExternalInput")
    pmTx = nc.dram_tensor("pmTx", [H + 1, NP1], fr, kind="ExternalInput")
    w2dt = f32r if USE_F32R else bf16
    w2blk = nc.dram_tensor("w2blk", [128, 5600], w2dt, kind="ExternalInput")
    iota = nc.dram_tensor("iota", [128, NP1], f32, kind="ExternalInput")
    tcol = nc.dram_tensor("tcol", [128, 1], f32, kind="ExternalInput")
    b2s = nc.dram_tensor("b2s", [128, 1], f32, kind="ExternalInput")
    ones128 = nc.dram_tensor("ones128", [128, 1], f32, kind="ExternalInput")
    pat4 = nc.dram_tensor("pat4", [100, 128], fr, kind="ExternalInput")
    out_sm = nc.dram_tensor("out_sm", [128, NP1], f32, kind="ExternalOutput")
    out_loss = nc.dram_tensor("out_loss", [1, 1], f32, kind="ExternalOutput")

    with tile.TileContext(nc) as tc:
        with (
            tc.tile_pool(name="consts", bufs=1) as cpool,
            tc.tile_pool(name="rpool", bufs=3) as rpool,
            tc.tile_pool(name="sgpool", bufs=3) as sgpool,
            tc.tile_pool(name="zpool", bufs=3) as zpool,
            tc.tile_pool(name="epool", bufs=1) as epool,
            tc.tile_pool(name="dram", bufs=1, space="DRAM") as dpool,
            tc.tile_pool(name="ps_s2b", bufs=1, space="PSUM") as ps_s2b_pool,
            tc.tile_pool(name="ps_b", bufs=1, space="PSUM") as ps_b_pool,
            tc.tile_pool(name="ps_sc", bufs=1, space="PSUM") as ps_sc_pool,
            tc.tile_pool(name="ps_l", bufs=1, space="PSUM") as ps_l_pool,
        ):
            # Dummy sigmoid so the ACT sigmoid table loads at t~0 instead of
            # on the critical path right before the first real sigmoid.
            dummy = cpool.tile([1, 1], f32, name="dummy")
            nc.vector.memset(dummy[:], 0.0)
            nc.scalar.activation(
                out=dummy[:], in_=dummy[:],
                func=mybir.ActivationFunctionType.Exp,
            )
            pat4_sb = cpool.tile([100, 128], fr, name="pat4_sb")
            nc.gpsimd.dma_start(out=pat4_sb[:], in_=pat4[:])

            # ---- load inputs into SBUF ----
            # s2b path on sync-HWDGE; B path + loop constants on ACT-HWDGE.
            pm_t = []
            wb_t = []
            for c, (k0, k1) in enumerate(KCH_B):
                pmc = cpool.tile([k1 - k0, NP1], fr, name=f"pmc{c}")
                nc.scalar.dma_start(out=pmc[:, 0:512], in_=pmTx[k0:k1, 0:512])
                nc.scalar.dma_start(out=pmc[:, 512:], in_=pmTx[k0:k1, 512:])
                pm_t.append(pmc)
                wbc = cpool.tile([k1 - k0, MID], fr, name=f"wbc{c}")
                nc.scalar.dma_start(out=wbc[:], in_=w1bTx[k0:k1, :])
                wb_t.append(wbc)
            st_t = []
            wa_t = []
            for c, (k0, k1) in enumerate(KCH_A):
                stc = cpool.tile([k1 - k0, NLOC], f32, name=f"stc{c}")
                nc.gpsimd.dma_start(out=stc[:], in_=sentT[k0:k1, :])
                st_t.append(stc)
                wac = cpool.tile([k1 - k0, MID], f32, name=f"wac{c}")
                nc.gpsimd.dma_start(out=wac[:], in_=w1aTr[k0:k1, :])
                wa_t.append(wac)
            # w2pack: per m_g a [128, 224] window whose 128-wide column slices
            # (shifted by 32*i_g) are the block-diagonal lhsT for (m_g, i_g).
            # Load in chunks so it doesn't monopolize the DMA engines.
            w2b_sb = cpool.tile([128, 5600], w2dt, name="w2b_sb")
            bulk_dmas = []
            bulk_dmas.append(nc.gpsimd.dma_start(out=w2b_sb[:, :224], in_=w2blk[:, :224]))
            for cw0 in range(224, 5600, 1120):
                cw1 = min(cw0 + 1120, 5600)
                bulk_dmas.append(
                    nc.gpsimd.dma_start(out=w2b_sb[:, cw0:cw1], in_=w2blk[:, cw0:cw1])
                )
            tcol_sb = cpool.tile([128, 1], f32, name="tcol_sb")
            nc.gpsimd.dma_start(out=tcol_sb[:], in_=tcol[:])
            b2s_sb = cpool.tile([128, 1], f32, name="b2s_sb")
            nc.gpsimd.dma_start(out=b2s_sb[:], in_=b2s[:])
            ones_sb = cpool.tile([128, 1], f32, name="ones_sb")
            nc.gpsimd.dma_start(out=ones_sb[:], in_=ones128[:])
            iota_sb = cpool.tile([128, NP1], f32, name="iota_sb")
            bulk_dmas.append(nc.gpsimd.dma_start(out=iota_sb[:, :512], in_=iota[:, :512]))
            bulk_dmas.append(nc.gpsimd.dma_start(out=iota_sb[:, 512:], in_=iota[:, 512:]))


            # ---- B bias arrangement: B[32*m_l + i_l, 25*i_g + m_g] = s1[i, m] ----
            # Groups sharing the same partitions+bank must not interleave on
            # the PE (start_tensor_calc clears has_written for the whole bank
            # row of the targeted partitions), so chain the 4 i_g groups of
            # each m_l strip explicitly.
            from concourse.tile import add_dep_helper as _adh

            ps_b = ps_b_pool.tile([128, MID], f32, name="ps_b")
            for m_l in range(4):
                prev_last = None
                for i_g in range(4):
                    for c in range(3):
                        mm = nc.tensor.matmul(
                            out=ps_b[32 * m_l : 32 * m_l + 32, 25 * i_g : 25 * i_g + 25],
                            lhsT=st_t[c][:, 32 * i_g : 32 * i_g + 32],
                            rhs=wa_t[c][:, 25 * m_l : 25 * m_l + 25],
                            start=(c == 0),
                            stop=(c == 2),
                            tile_position=(0, 32 * m_l),
                            skip_group_check=True,
                        )
                        if c == 0 and prev_last is not None:
                            _adh(mm.ins, prev_last, reason="chain psum groups")
                    prev_last = mm.ins
            B_sb = cpool.tile([128, MID], f32, name="B_sb")
            nc.vector.tensor_copy(out=B_sb[:], in_=ps_b[:])


            def _n1cast(ap):
                # f32r matmuls with a 1-wide moving dim are invalid ISA; read
                # the (already-rounded) bits as plain f32 for those.
                return ap.bitcast(f32) if ap.dtype == mybir.dt.float32r else ap
            # ---- s2b = W1b @ pm.T + b1  -> [100, 1025] (m on partitions) ----
            ps_s2b = ps_s2b_pool.tile(
                [MID, NP1], f32, name="ps_s2b", padded_shape=[MID, 1536]
            )
            for n0, n1 in NCHUNKS:
                for c in range(3):
                    lhsT_mm = wb_t[c][:]
                    rhs_mm = pm_t[c][:, n0:n1]
                    if n1 - n0 < 256:
                        lhsT_mm = _n1cast(lhsT_mm)
                        rhs_mm = _n1cast(rhs_mm)
                    _mm = nc.tensor.matmul(
                        out=ps_s2b[:, n0:n1],
                        lhsT=lhsT_mm,
                        rhs=rhs_mm,
                        start=(c == 0),
                        stop=(c == 2),
                        skip_group_check=True,
                    )
                    if (n0, c) == (0, 0):
                        first_s2b_mm = _mm
            from concourse.tile import add_dep_helper as _adh0
            _adh0(bulk_dmas[0].ins, first_s2b_mm.ins, reason="keep startup DMA path clear")
            s2b_sb = cpool.tile([MID, NP1 + 1], fr, name="s2b_sb")
            for n0, n1 in NCHUNKS:
                nc.vector.tensor_copy(out=s2b_sb[:, n0:n1], in_=ps_s2b[:, n0:n1])
            # pad column so 1026-wide broadcast reads stay in-bounds/defined
            nc.vector.tensor_copy(out=s2b_sb[:, 1025:1026], in_=ps_s2b[:, 1024:1025])
            s2b_dram = dpool.tile([MID, NP1 + 1], fr, name="s2b_dram")
            nc.sync.dma_start(out=s2b_dram[:, 0:512], in_=s2b_sb[:, 0:512])
            nc.sync.dma_start(out=s2b_dram[:, 512:1024], in_=s2b_sb[:, 512:1024])
            nc.sync.dma_start(out=s2b_dram[:, 1024:1026], in_=s2b_sb[:, 1024:1026])

            # one-hot(target) — independent of the main loop; compute early on
            # the otherwise-idle DVE.
            oh = epool.tile([128, NP1], f32, name="oh")
            nc.vector.tensor_scalar(
                out=oh[:],
                in0=iota_sb[:],
                scalar1=tcol_sb[:, 0:1],
                scalar2=None,
                op0=mybir.AluOpType.is_equal,
            )

            # ---- main loop: sigmoid + W2 contraction ----
            # One PSUM tile per N-chunk (each exactly one bank per partition) so
            # the 4 i-group accumulation regions stay bank-disjoint.
            ps_sc0 = ps_sc_pool.tile([128, 512], f32, name="ps_sc0")
            ps_sc1 = ps_sc_pool.tile([128, 512], f32, name="ps_sc1")
            ps_sc2 = ps_sc_pool.tile([128, 1], f32, name="ps_sc2")
            ps_sc_tiles = [ps_sc0, ps_sc1, ps_sc2]
            import concourse.bass as _bass

            # m_g in {0, 8} have 32-aligned partition bases in s2b, so their
            # replicated R tiles can be built by a K=4 pattern matmul into
            # PSUM straight from SBUF — the first sigmoids don't wait for the
            # s2b -> DRAM -> broadcast-DMA roundtrip.
            MG_PE = (0, 8)
            mg_order = list(MG_PE) + [m for m in range(25) if m not in MG_PE]
            W = NP1 + 1  # 1026: even inner dim keeps DVE tensor_scalar in 2x mode
            for m_g in mg_order:
                if m_g in MG_PE:
                    # PSUM-built R; keep the per-i_g biased-tanh form so the
                    # first sigmoids don't wait on DVE bias-adds.
                    R = ps_s2b_pool.tile(
                        [128, NP1], f32, name=f"r_ps{m_g}", tag="ps_s2b",
                        padded_shape=[128, 1536],
                    )
                    pb = 4 * m_g // 32 * 32
                    for n0, n1 in NCHUNKS:
                        lhsT_mm = pat4_sb[pb : pb + 4, :]
                        rhs_mm = s2b_sb[4 * m_g : 4 * m_g + 4, n0:n1]
                        if n1 - n0 < 256:
                            lhsT_mm = _n1cast(lhsT_mm)
                            rhs_mm = _n1cast(rhs_mm)
                        # skip_group_check: the sim's zero-region shadow uses a
                        # bank-sized partition stride; on HW the clear stays in
                        # this tile's own banks.
                        nc.tensor.matmul(
                            out=R[:, n0:n1],
                            lhsT=lhsT_mm,
                            rhs=rhs_mm,
                            tile_position=(4 * m_g // 32 * 32, 0),
                            skip_group_check=True,
                        )
                    sgs = []
                    for i_g in range(4):
                        sg = sgpool.tile([128, W], fr if USE_F32R else bf16, name="sg", tag="sgn")
                        nc.scalar.activation(
                            out=sg[:, 0:NP1],
                            in_=R[:].bitcast(f32),
                            func=mybir.ActivationFunctionType.Tanh,
                            bias=B_sb[:, 25 * i_g + m_g : 25 * i_g + m_g + 1],
                            scale=1.0,
                        )
                        sgs.append((sg, 0))
                else:
                    # wide form: DVE adds the per-(i_g) bias into a 4-block z
                    # tile; ONE tanh instruction covers all 4 i-groups.
                    R = rpool.tile([128, W], fr, name="R")
                    src = _bass.AP(
                        tensor=s2b_dram.tensor,
                        offset=s2b_dram.offset + (4 * m_g) * W,
                        ap=[[W, 4], [0, 32], [1, W]],
                    )
                    _rdma = nc.sync.dma_start(out=R[:], in_=src)
                    if m_g == mg_order[len(MG_PE)]:
                        first_r_dma = _rdma
                    zw = zpool.tile([128, 4 * W], f32, name="zw")
                    for i_g in range(4):
                        nc.vector.tensor_scalar_add(
                            out=zw[:, i_g * W : (i_g + 1) * W],
                            in0=R[:].bitcast(f32),
                            scalar1=B_sb[:, 25 * i_g + m_g : 25 * i_g + m_g + 1],
                        )
                    sgw = sgpool.tile([128, 4 * W], fr if USE_F32R else bf16, name="sgw", tag="sgw")
                    nc.scalar.activation(
                        out=sgw[:],
                        in_=zw[:],
                        func=mybir.ActivationFunctionType.Tanh,
                        bias=0.0,
                        scale=1.0,
                    )
                    sgs = [(sgw, i_g * W) for i_g in range(4)]
                for i_g in range(4):
                    sg, soff = sgs[i_g]
                    woff = 224 * m_g + 96 - 32 * i_g
                    for ci, (n0, n1) in enumerate(NCHUNKS):
                        lhsT_mm = w2b_sb[:, woff : woff + 128]
                        rhs_mm = sg[:, soff + n0 : soff + n1]
                        if n1 - n0 < 256:
                            lhsT_mm = _n1cast(lhsT_mm)
                            rhs_mm = _n1cast(rhs_mm)
                        nc.tensor.matmul(
                            out=ps_sc_tiles[ci][:, : n1 - n0],
                            lhsT=lhsT_mm,
                            rhs=rhs_mm,
                            start=(m_g == mg_order[0] and i_g == 0),
                            stop=(m_g == mg_order[-1] and i_g == 3),
                            skip_group_check=True,
                        )

            for bd in bulk_dmas[1:]:
                _adh0(bd.ins, first_r_dma.ins, reason="bulk loads after first R stream")

            # ---- epilogue: scores -> loss partial + softmax ----
            scores_sb = epool.tile([128, NP1], f32, name="scores_sb")
            for ci, (n0, n1) in enumerate(NCHUNKS):
                nc.vector.tensor_scalar_add(
                    out=scores_sb[:, n0:n1],
                    in0=ps_sc_tiles[ci][:, : n1 - n0],
                    scalar1=b2s_sb[:, 0:1],
                )

            dt_ = epool.tile([128, NP1], f32, name="dt_")
            nc.vector.tensor_sub(dt_[:], scores_sb[:], oh[:])
            lp = epool.tile([128, 1], f32, name="lp")
            nc.vector.tensor_reduce(
                out=lp[:],
                in_=dt_[:],
                axis=mybir.AxisListType.X,
                op=mybir.AluOpType.add,
                apply_absolute_value=True,
            )
            ps_l = ps_l_pool.tile([1, 1], f32, name="ps_l")
            nc.tensor.matmul(
                out=ps_l[:], lhsT=lp[:], rhs=ones_sb[:], skip_group_check=True
            )
            loss_sb = epool.tile([1, 1], f32, name="loss_sb")
            nc.vector.tensor_copy(out=loss_sb[:], in_=ps_l[:])
            nc.scalar.dma_start(out=out_loss[:], in_=loss_sb[:])

            # Softmax ignores the constant b2 shift (softmax(s+b2)==softmax(s))
            # and scores are bounded (|s| <= sum|W2| ~ 6), so exp can read the
            # raw PSUM score chunks directly with bias=0 — each chunk's exp
            # fires as soon as that chunk's accumulation stops, off the
            # bias-add critical path.
            ex = epool.tile([128, NP1], f32, name="ex")
            es3 = epool.tile([128, 3], f32, name="es3")
            for ci, (n0, n1) in enumerate(NCHUNKS):
                nc.scalar.activation(
                    out=ex[:, n0:n1],
                    in_=ps_sc_tiles[ci][:, : n1 - n0],
                    func=mybir.ActivationFunctionType.Exp,
                    bias=0.0,
                    scale=1.0,
                    accum_out=es3[:, ci : ci + 1],
                )
            es = epool.tile([128, 1], f32, name="es")
            nc.vector.reduce_sum(out=es[:], in_=es3[:], axis=mybir.AxisListType.X)
            rec = epool.tile([128, 1], f32, name="rec")
            nc.vector.reciprocal(out=rec[:], in_=es[:])
            sm = epool.tile([128, NP1], f32, name="sm")
            nc.vector.tensor_scalar_mul(
                out=sm[:, 0:512], in0=ex[:, 0:512], scalar1=rec[:, 0:1]
            )
            nc.scalar.dma_start(out=out_sm[:, 0:512], in_=sm[:, 0:512])
            nc.vector.tensor_scalar_mul(
                out=sm[:, 512:], in0=ex[:, 512:], scalar1=rec[:, 0:1]
            )
            nc.scalar.dma_start(out=out_sm[:, 512:], in_=sm[:, 512:])

    return nc


def get_program(for_sim=False):
    global _PROGRAM
    if for_sim:
        nc = _build_program()
        nc.compile()
        return nc
    if _PROGRAM is None:
        nc = _build_program()
        nc.finalize()
        _PROGRAM = nc
    return _PROGRAM


def prep_inputs(sentence, target_scores, W1, b1, W2, b2):
    """Host-side layout prep + sharding. Returns in_maps for the 8 cores."""
    sentence = np.ascontiguousarray(np.asarray(sentence, dtype=np.float32))
    target_scores = np.asarray(target_scores).astype(np.int64)
    W1 = np.asarray(W1, dtype=np.float32)
    b1 = np.asarray(b1, dtype=np.float32)
    W2 = np.asarray(W2, dtype=np.float32)
    b2 = np.asarray(b2, dtype=np.float32)

    sentT_all = np.ascontiguousarray(sentence.T)  # [300, 1024]
    pmT = np.concatenate(
        [np.zeros((1, H), np.float32), sentence], axis=0
    ).T  # [300, 1025]
    pmTx = np.ascontiguousarray(
        np.concatenate([pmT, np.ones((1, NP1), np.float32)], axis=0)
    )  # [301, 1025]
    # column k of w1aTr (k = 25*m_l + m_g) = W1a row (4*m_g + m_l)
    idx = np.array([4 * m_g + m_l for m_l in range(4) for m_g in range(25)])
    # halved: sigmoid(z) computed on-device as 0.5 + 0.5*tanh(z/2)
    w1aTr = np.ascontiguousarray(W1[idx, :H].T) * 0.5  # [300, 100]
    w1bTx = np.ascontiguousarray(
        np.concatenate([W1[:, H:].T, b1[None, :]], axis=0)
    ) * 0.5  # [301, 100]

    # Packed W2 block weights: per m_g one [128, 224] window with nonzeros at
    # column 96 + p%32, value W2[4*m_g + p//32].  The lhsT for (m_g, i_g) is
    # the 128-column slice at offset 224*m_g + 96 - 32*i_g, which places the
    # nonzero at local column 32*i_g + p%32 — i.e. out row 32*i_g + p%32 —
    # summing the four m_l lanes into each i's score.
    w2blk = np.zeros((128, 5600), np.float32)
    P = np.arange(128)
    for m_g in range(25):
        w2blk[P, 224 * m_g + 96 + (P % 32)] = 0.5 * W2[0, 4 * m_g + P // 32]
    if not USE_F32R:
        w2blk = w2blk.astype(ml_dtypes.bfloat16)

    iota = np.ascontiguousarray(
        np.broadcast_to(np.arange(NP1, dtype=np.float32), (128, NP1))
    )
    b2s = np.full((128, 1), b2.reshape(-1)[0] + 0.5 * W2.sum(), np.float32)
    pat4 = np.zeros((100, 128), np.float32)
    pat4[np.arange(128) // 32, np.arange(128)] = 1.0
    pat4[32:36] = pat4[0:4]
    pat4[64:68] = pat4[0:4]
    pat4[96:100] = pat4[0:4]
    ones = np.ones((128, 1), np.float32)

    in_maps = []
    for k in range(NCORES):
        sl = slice(k * NLOC, (k + 1) * NLOC)
        in_maps.append(
            {
                "sentT": np.ascontiguousarray(sentT_all[:, sl]),
                "w1aTr": w1aTr,
                "w1bTx": w1bTx,
                "pmTx": pmTx,
                "w2blk": w2blk,
                "iota": iota,
                "tcol": target_scores[sl].astype(np.float32).reshape(NLOC, 1),
                "b2s": b2s,
                "ones128": ones,
                "pat4": pat4,
            }
        )
    return in_maps


def assemble_outputs(results):
    """results: list of 8 dicts with out_sm [128,1025] and out_loss [1,1]."""
    score_matrix = np.concatenate([r["out_sm"] for r in results], axis=0)
    loss_sum = sum(float(r["out_loss"][0, 0]) for r in results)
    loss = np.float32(loss_sum / (N * NP1))
    return loss, score_matrix


def run(inputs, **kwargs):
    from concourse.bass_utils import run_bass_kernel_spmd

    nc = get_program()
    in_maps = prep_inputs(
        inputs["sentence"],
        inputs["target_scores"],
        inputs["W1"],
        inputs["b1"],
        inputs["W2"],
        inputs["b2"],
    )
    res = run_bass_kernel_spmd(nc, in_maps, core_ids=list(range(NCORES)), **kwargs)
    return assemble_outputs(res.results), res


def kernel(**inputs):
    out, _ = run(inputs)
    return out


# revision 59
# speedup vs baseline: 1.0054x; 1.0054x over previous
"""Trainium2 Bass kernel for nn_DependencyParser.

Math (reference):
  pm = [zeros(1,H); sentence]                         # [1025, 300]
  s1 = sentence @ W1[:, :H].T                          # [1024, 100]
  s2 = pm @ W1[:, H:].T                                # [1025, 100]
  hidden = sigmoid(s1[i,m] + s2[j,m] + b1[m])          # [1024, 1025, 100]
  scores = hidden @ W2.T + b2                          # [1024, 1025]
  loss = mean |scores - onehot(target)|
  out = softmax(scores, axis=1)

Sharding: rows i split across 8 cores (128 rows each); weights replicated.

Per-core layout ("packed" scheme): each ACT instruction evaluates sigmoid for
32 i's x 4 m's using all 128 partitions: partition p = 32*m_l + i_l covers
(i = 32*i_g + i_l, m = 4*m_g + m_l).  The input tile R holds s2b rows
[4*m_g .. 4*m_g+4) each replicated 32x — built by a broadcast DMA from a DRAM
staging copy of s2b (for m_g in MG_PE, by a K=4 pattern matmul into PSUM so
the first sigmoids skip the DRAM roundtrip).  The per-partition ACT bias
carries s1[i, m] (precomputed into B via 48 small matmuls).  The W2
contraction is one M=128 matmul per (m_g, i_g) with a sparse block weight
matrix (a sliding 128-column window of w2blk), PSUM-accumulated over all 100
(m_g, i_g) steps into three bank-sized score chunks.  Matmuls use float32r
(TF32-like, full rate, ~2^-11 rounding) so the sigmoid outputs stay in fp32.

The sigmoid itself is computed as 0.5 + 0.5*tanh(z/2) with the affine folded
into host-side constants (W1/b1 halved, W2 halved, b2 shifted by 0.5*sum(W2))
— tanh shares the ACT table set with exp, so the softmax needs no table-set
switch.  For DRAM-fed m-groups the per-i-group s1 bias is added by the (idle)
VectorEngine into a 4-block-wide z tile, so ONE tanh instruction covers all
four i-groups ([128 x 4104]) — amortizing the ~224-cycle per-instruction ACT
overhead 4x (ACT busy ~94us instead of ~107us).  Rows are padded to 1026 so
the DVE bias-adds stay in their 2x perf mode.

The ACT engine is the bottleneck by design; PE (~56us), DVE (~60us) and DMA
(~60us) all hide underneath.  Cost-model total ~120.6us/core: ~13us pipeline
fill + ~95us tanh span + ~9us softmax/loss tail.
"""

import sys

if "/opt/trn_rl_repo" not in sys.path:
    sys.path.insert(0, "/opt/trn_rl_repo")

import numpy as np
import ml_dtypes

N = 1024
H = 300
MID = 100
NP1 = 1025
NCORES = 8
NLOC = N // NCORES  # 128 rows per core
NCHUNKS = [(0, 512), (512, 1024), (1024, 1025)]  # PSUM-bank-sized N chunks
KCH_A = [(0, 128), (128, 256), (256, 300)]  # K chunks for s1 (300)
KCH_B = [(0, 128), (128, 256), (256, 301)]  # K chunks for s2 (300 + bias row)

USE_F32R = True  # float32r (TF32-like) matmuls: full-rate + better precision than bf16
_PROGRAM = None


def _build_program():
    import concourse.bass as bass
    import concourse.tile as tile
    from concourse import bacc, mybir

    f32 = mybir.dt.float32
    f32r = mybir.dt.float32r
    bf16 = mybir.dt.bfloat16
    fr = f32r if USE_F32R else f32

    nc = bacc.Bacc(trn_type="TRN2")

    # ---- I/O ----
    sentT = nc.dram_tensor("sentT", [H, NLOC], f32, kind="ExternalInput")
    w1aTr = nc.dram_tensor("w1aTr", [H, MID], f32, kind="ExternalInput")
    w1bTx = nc.dram_tensor("w1bTx", [H + 1, MID], fr, kind="ExternalInput")
    pmTx = nc.dram_tensor("pmTx", [H + 1, NP1], fr, kind="ExternalInput")
    w2dt = f32r if USE_F32R else bf16
    w2blk = nc.dram_tensor("w2blk", [128, 5600], w2dt, kind="ExternalInput")
    iota = nc.dram_tensor("iota", [128, NP1], f32, kind="ExternalInput")
    tcol = nc.dram_tensor("tcol", [128, 1], f32, kind="ExternalInput")
    b2s = nc.dram_tensor("b2s", [128, 1], f32, kind="ExternalInput")
    ones128 = nc.dram_tensor("ones128", [128, 1], f32, kind="ExternalInput")
    pat4 = nc.dram_tensor("pat4", [100, 128], fr, kind="ExternalInput")
    out_sm = nc.dram_tensor("out_sm", [128, NP1], f32, kind="ExternalOutput")
    out_loss = nc.dram_tensor("out_loss", [1, 1], f32, kind="ExternalOutput")

    with tile.TileContext(nc) as tc:
        with (
            tc.tile_pool(name="consts", bufs=1) as cpool,
            tc.tile_pool(name="rpool", bufs=3) as rpool,
            tc.tile_pool(name="sgpool", bufs=4) as sgpool,
            tc.tile_pool(name="zpool", bufs=3) as zpool,
            tc.tile_pool(name="epool", bufs=1) as epool,
            tc.tile_pool(name="dram", bufs=1, space="DRAM") as dpool,
            tc.tile_pool(name="ps_s2b", bufs=1, space="PSUM") as ps_s2b_pool,
            tc.tile_pool(name="ps_b", bufs=1, space="PSUM") as ps_b_pool,
            tc.tile_pool(name="ps_sc", bufs=1, space="PSUM") as ps_sc_pool,
            tc.tile_pool(name="ps_l", bufs=1, space="PSUM") as ps_l_pool,
        ):
            # Dummy sigmoid so the ACT sigmoid table loads at t~0 instead of
            # on the critical path right before the first real sigmoid.
            dummy = cpool.tile([1, 1], f32, name="dummy")
            nc.vector.memset(dummy[:], 0.0)
            nc.scalar.activation(
                out=dummy[:], in_=dummy[:],
                func=mybir.ActivationFunctionType.Exp,
            )
            pat4_sb = cpool.tile([100, 128], fr, name="pat4_sb")
            nc.gpsimd.dma_start(out=pat4_sb[:], in_=pat4[:])

            # ---- load inputs into SBUF ----
            # s2b path on sync-HWDGE; B path + loop constants on ACT-HWDGE.
            pm_t = []
            wb_t = []
            for c, (k0, k1) in enumerate(KCH_B):
                pmc = cpool.tile([k1 - k0, NP1], fr, name=f"pmc{c}")
                nc.scalar.dma_start(out=pmc[:, 0:512], in_=pmTx[k0:k1, 0:512])
                nc.scalar.dma_start(out=pmc[:, 512:], in_=pmTx[k0:k1, 512:])
                pm_t.append(pmc)
                wbc = cpool.tile([k1 - k0, MID], fr, name=f"wbc{c}")
                nc.scalar.dma_start(out=wbc[:], in_=w1bTx[k0:k1, :])
                wb_t.append(wbc)
            st_t = []
            wa_t = []
            for c, (k0, k1) in enumerate(KCH_A):
                stc = cpool.tile([k1 - k0, NLOC], f32, name=f"stc{c}")
                nc.gpsimd.dma_start(out=stc[:], in_=sentT[k0:k1, :])
                st_t.append(stc)
                wac = cpool.tile([k1 - k0, MID], f32, name=f"wac{c}")
                nc.gpsimd.dma_start(out=wac[:], in_=w1aTr[k0:k1, :])
                wa_t.append(wac)
            # w2pack: per m_g a [128, 224] window whose 128-wide column slices
            # (shifted by 32*i_g) are the block-diagonal lhsT for (m_g, i_g).
            # Load in chunks so it doesn't monopolize the DMA engines.
            w2b_sb = cpool.tile([128, 5600], w2dt, name="w2b_sb")
            bulk_dmas = []
            bulk_dmas.append(nc.gpsimd.dma_start(out=w2b_sb[:, :224], in_=w2blk[:, :224]))
            for cw0 in range(224, 5600, 1120):
                cw1 = min(cw0 + 1120, 5600)
                bulk_dmas.append(
                    nc.gpsimd.dma_start(out=w2b_sb[:, cw0:cw1], in_=w2blk[:, cw0:cw1])
                )
            tcol_sb = cpool.tile([128, 1], f32, name="tcol_sb")
            nc.gpsimd.dma_start(out=tcol_sb[:], in_=tcol[:])
            b2s_sb = cpool.tile([128, 1], f32, name="b2s_sb")
            nc.gpsimd.dma_start(out=b2s_sb[:], in_=b2s[:])
            ones_sb = cpool.tile([128, 1], f32, name="ones_sb")
            nc.gpsimd.dma_start(out=ones_sb[:], in_=ones128[:])
            iota_sb = cpool.tile([128, NP1], f32, name="iota_sb")
            bulk_dmas.append(nc.gpsimd.dma_start(out=iota_sb[:, :512], in_=iota[:, :512]))
            bulk_dmas.append(nc.gpsimd.dma_start(out=iota_sb[:, 512:], in_=iota[:, 512:]))


            # ---- B bias arrangement: B[32*m_l + i_l, 25*i_g + m_g] = s1[i, m] ----
            # Groups sharing the same partitions+bank must not interleave on
            # the PE (start_tensor_calc clears has_written for the whole bank
            # row of the targeted partitions), so chain the 4 i_g groups of
            # each m_l strip explicitly.
            from concourse.tile import add_dep_helper as _adh

            ps_b = ps_b_pool.tile([128, MID], f32, name="ps_b")
            for m_l in range(4):
                prev_last = None
                for i_g in range(4):
                    for c in range(3):
                        mm = nc.tensor.matmul(
                            out=ps_b[32 * m_l : 32 * m_l + 32, 25 * i_g : 25 * i_g + 25],
                            lhsT=st_t[c][:, 32 * i_g : 32 * i_g + 32],
                            rhs=wa_t[c][:, 25 * m_l : 25 * m_l + 25],
                            start=(c == 0),
                            stop=(c == 2),
                            tile_position=(0, 32 * m_l),
                            skip_group_check=True,
                        )
                        if c == 0 and prev_last is not None:
                            _adh(mm.ins, prev_last, reason="chain psum groups")
                    prev_last = mm.ins
            B_sb = cpool.tile([128, MID], f32, name="B_sb")
            nc.vector.tensor_copy(out=B_sb[:], in_=ps_b[:])


            def _n1cast(ap):
                # f32r matmuls with a 1-wide moving dim are invalid ISA; read
                # the (already-rounded) bits as plain f32 for those.
                return ap.bitcast(f32) if ap.dtype == mybir.dt.float32r else ap
            # ---- s2b = W1b @ pm.T + b1  -> [100, 1025] (m on partitions) ----
            ps_s2b = ps_s2b_pool.tile(
                [MID, NP1], f32, name="ps_s2b", padded_shape=[MID, 1536]
            )
            for n0, n1 in NCHUNKS:
                for c in range(3):
                    lhsT_mm = wb_t[c][:]
                    rhs_mm = pm_t[c][:, n0:n1]
                    if n1 - n0 < 256:
                        lhsT_mm = _n1cast(lhsT_mm)
                        rhs_mm = _n1cast(rhs_mm)
                    _mm = nc.tensor.matmul(
                        out=ps_s2b[:, n0:n1],
                        lhsT=lhsT_mm,
                        rhs=rhs_mm,
                        start=(c == 0),
                        stop=(c == 2),
                        skip_group_check=True,
                    )
                    if (n0, c) == (0, 0):
                        first_s2b_mm = _mm
            from concourse.tile import add_dep_helper as _adh0
            _adh0(bulk_dmas[0].ins, first_s2b_mm.ins, reason="keep startup DMA path clear")
            s2b_sb = cpool.tile([MID, NP1 + 1], fr, name="s2b_sb")
            for n0, n1 in NCHUNKS:
                nc.vector.tensor_copy(out=s2b_sb[:, n0:n1], in_=ps_s2b[:, n0:n1])
            # pad column so 1026-wide broadcast reads stay in-bounds/defined
            nc.vector.tensor_copy(out=s2b_sb[:, 1025:1026], in_=ps_s2b[:, 1024:1025])
            s2b_dram = dpool.tile([MID, NP1 + 1], fr, name="s2b_dram")
            nc.sync.dma_start(out=s2b_dram[:, 0:512], in_=s2b_sb[:, 0:512])
            nc.sync.dma_start(out=s2b_dram[:, 512:1024], in_=s2b_sb[:, 512:1024])
            nc.sync.dma_start(out=s2b_dram[:, 1024:1026], in_=s2b_sb[:, 1024:1026])

            # one-hot(target) — independent of the main loop; compute early on
            # the otherwise-idle DVE.
            oh = epool.tile([128, NP1], f32, name="oh")
            nc.vector.tensor_scalar(
                out=oh[:],
                in0=iota_sb[:],
                scalar1=tcol_sb[:, 0:1],
                scalar2=None,
                op0=mybir.AluOpType.is_equal,
            )

            # ---- main loop: sigmoid + W2 contraction ----
            # One PSUM tile per N-chunk (each exactly one bank per partition) so
            # the 4 i-group accumulation regions stay bank-disjoint.
            ps_sc0 = ps_sc_pool.tile([128, 512], f32, name="ps_sc0")
            ps_sc1 = ps_sc_pool.tile([128, 512], f32, name="ps_sc1")
            ps_sc2 = ps_sc_pool.tile([128, 1], f32, name="ps_sc2")
            ps_sc_tiles = [ps_sc0, ps_sc1, ps_sc2]
            import concourse.bass as _bass

            # m_g in {0, 8} have 32-aligned partition bases in s2b, so their
            # replicated R tiles can be built by a K=4 pattern matmul into
            # PSUM straight from SBUF — the first sigmoids don't wait for the
            # s2b -> DRAM -> broadcast-DMA roundtrip.
            MG_PE = (0, 8)
            mg_order = list(MG_PE) + [m for m in range(25) if m not in MG_PE]
            W = NP1 + 1  # 1026: even inner dim keeps DVE tensor_scalar in 2x mode
            for m_g in mg_order:
                if m_g in MG_PE:
                    # PSUM-built R; keep the per-i_g biased-tanh form so the
                    # first sigmoids don't wait on DVE bias-adds.
                    R = ps_s2b_pool.tile(
                        [128, NP1], f32, name=f"r_ps{m_g}", tag="ps_s2b",
                        padded_shape=[128, 1536],
                    )
                    pb = 4 * m_g // 32 * 32
                    for n0, n1 in NCHUNKS:
                        lhsT_mm = pat4_sb[pb : pb + 4, :]
                        rhs_mm = s2b_sb[4 * m_g : 4 * m_g + 4, n0:n1]
                        if n1 - n0 < 256:
                            lhsT_mm = _n1cast(lhsT_mm)
                            rhs_mm = _n1cast(rhs_mm)
                        # skip_group_check: the sim's zero-region shadow uses a
                        # bank-sized partition stride; on HW the clear stays in
                        # this tile's own banks.
                        nc.tensor.matmul(
                            out=R[:, n0:n1],
                            lhsT=lhsT_mm,
                            rhs=rhs_mm,
                            tile_position=(4 * m_g // 32 * 32, 0),
                            skip_group_check=True,
                        )
                    sgs = []
                    for i_g in range(4):
                        sg = sgpool.tile([128, W], fr if USE_F32R else bf16, name="sg", tag="sgn")
                        nc.scalar.activation(
                            out=sg[:, 0:NP1],
                            in_=R[:].bitcast(f32),
                            func=mybir.ActivationFunctionType.Tanh,
                            bias=B_sb[:, 25 * i_g + m_g : 25 * i_g + m_g + 1],
                            scale=1.0,
                        )
                        sgs.append((sg, 0))
                else:
                    # wide form: DVE adds the per-(i_g) bias into a 4-block z
                    # tile; ONE tanh instruction covers all 4 i-groups.
                    R = rpool.tile([128, W], fr, name="R")
                    src = _bass.AP(
                        tensor=s2b_dram.tensor,
                        offset=s2b_dram.offset + (4 * m_g) * W,
                        ap=[[W, 4], [0, 32], [1, W]],
                    )
                    _rdma = nc.sync.dma_start(out=R[:], in_=src)
                    if m_g == mg_order[len(MG_PE)]:
                        first_r_dma = _rdma
                    zw = zpool.tile([128, 4 * W], f32, name="zw")
                    for i_g in range(4):
                        nc.vector.tensor_scalar_add(
                            out=zw[:, i_g * W : (i_g + 1) * W],
                            in0=R[:].bitcast(f32),
                            scalar1=B_sb[:, 25 * i_g + m_g : 25 * i_g + m_g + 1],
                        )
                    sgw = sgpool.tile([128, 4 * W], fr if USE_F32R else bf16, name="sgw", tag="sgw")
                    nc.scalar.activation(
                        out=sgw[:],
                        in_=zw[:],
                        func=mybir.ActivationFunctionType.Tanh,
                        bias=0.0,
                        scale=1.0,
                    )
                    sgs = [(sgw, i_g * W) for i_g in range(4)]
                for i_g in range(4):
                    sg, soff = sgs[i_g]
                    woff = 224 * m_g + 96 - 32 * i_g
                    for ci, (n0, n1) in enumerate(NCHUNKS):
                        lhsT_mm = w2b_sb[:, woff : woff + 128]
                        rhs_mm = sg[:, soff + n0 : soff + n1]
                        if n1 - n0 < 256:
                            lhsT_mm = _n1cast(lhsT_mm)
                            rhs_mm = _n1cast(rhs_mm)
                        nc.tensor.matmul(
                            out=ps_sc_tiles[ci][:, : n1 - n0],
                            lhsT=lhsT_mm,
                            rhs=rhs_mm,
                            start=(m_g == mg_order[0] and i_g == 0),
                            stop=(m_g == mg_order[-1] and i_g == 3),
                            skip_group_check=True,
                        )

            for bd in bulk_dmas[1:]:
                _adh0(bd.ins, first_r_dma.ins, reason="bulk loads after first R stream")

            # ---- epilogue: scores -> loss partial + softmax ----
            scores_sb = epool.tile([128, NP1], f32, name="scores_sb")
            for ci, (n0, n1) in enumerate(NCHUNKS):
                nc.vector.tensor_scalar_add(
                    out=scores_sb[:, n0:n1],
                    in0=ps_sc_tiles[ci][:, : n1 - n0],
                    scalar1=b2s_sb[:, 0:1],
                )

            dt_ = epool.tile([128, NP1], f32, name="dt_")
            nc.vector.tensor_sub(dt_[:], scores_sb[:], oh[:])
            lp = epool.tile([128, 1], f32, name="lp")
            nc.vector.tensor_reduce(
                out=lp[:],
                in_=dt_[:],
                axis=mybir.AxisListType.X,
                op=mybir.AluOpType.add,
                apply_absolute_value=True,
            )
            ps_l = ps_l_pool.tile([1, 1], f32, name="ps_l")
            nc.tensor.matmul(
                out=ps_l[:], lhsT=lp[:], rhs=ones_sb[:], skip_group_check=True
            )
            loss_sb = epool.tile([1, 1], f32, name="loss_sb")
            nc.vector.tensor_copy(out=loss_sb[:], in_=ps_l[:])
            nc.scalar.dma_start(out=out_loss[:], in_=loss_sb[:])

            # Softmax ignores the constant b2 shift (softmax(s+b2)==softmax(s))
            # and scores are bounded (|s| <= sum|W2| ~ 6), so exp can read the
            # raw PSUM score chunks directly with bias=0 — each chunk's exp
            # fires as soon as that chunk's accumulation stops, off the
            # bias-add critical path.
            ex = epool.tile([128, NP1], f32, name="ex")
            es3 = epool.tile([128, 3], f32, name="es3")
            for ci, (n0, n1) in enumerate(NCHUNKS):
                nc.scalar.activation(
                    out=ex[:, n0:n1],
                    in_=ps_sc_tiles[ci][:, : n1 - n0],
                    func=mybir.ActivationFunctionType.Exp,
                    bias=0.0,
                    scale=1.0,
                    accum_out=es3[:, ci : ci + 1],
                )
            es = epool.tile([128, 1], f32, name="es")
            nc.vector.reduce_sum(out=es[:], in_=es3[:], axis=mybir.AxisListType.X)
            rec = epool.tile([128, 1], f32, name="rec")
            nc.vector.reciprocal(out=rec[:], in_=es[:])
            sm = epool.tile([128, NP1], f32, name="sm")
            nc.vector.tensor_scalar_mul(
                out=sm[:, 0:512], in0=ex[:, 0:512], scalar1=rec[:, 0:1]
            )
            nc.scalar.dma_start(out=out_sm[:, 0:512], in_=sm[:, 0:512])
            nc.vector.tensor_scalar_mul(
                out=sm[:, 512:], in0=ex[:, 512:], scalar1=rec[:, 0:1]
            )
            nc.scalar.dma_start(out=out_sm[:, 512:], in_=sm[:, 512:])

    return nc


def get_program(for_sim=False):
    global _PROGRAM
    if for_sim:
        nc = _build_program()
        nc.compile()
        return nc
    if _PROGRAM is None:
        nc = _build_program()
        nc.finalize()
        _PROGRAM = nc
    return _PROGRAM


def prep_inputs(sentence, target_scores, W1, b1, W2, b2):
    """Host-side layout prep + sharding. Returns in_maps for the 8 cores."""
    sentence = np.ascontiguousarray(np.asarray(sentence, dtype=np.float32))
    target_scores = np.asarray(target_scores).astype(np.int64)
    W1 = np.asarray(W1, dtype=np.float32)
    b1 = np.asarray(b1, dtype=np.float32)
    W2 = np.asarray(W2, dtype=np.float32)
    b2 = np.asarray(b2, dtype=np.float32)

    sentT_all = np.ascontiguousarray(sentence.T)  # [300, 1024]
    pmT = np.concatenate(
        [np.zeros((1, H), np.float32), sentence], axis=0
    ).T  # [300, 1025]
    pmTx = np.ascontiguousarray(
        np.concatenate([pmT, np.ones((1, NP1), np.float32)], axis=0)
    )  # [301, 1025]
    # column k of w1aTr (k = 25*m_l + m_g) = W1a row (4*m_g + m_l)
    idx = np.array([4 * m_g + m_l for m_l in range(4) for m_g in range(25)])
    # halved: sigmoid(z) computed on-device as 0.5 + 0.5*tanh(z/2)
    w1aTr = np.ascontiguousarray(W1[idx, :H].T) * 0.5  # [300, 100]
    w1bTx = np.ascontiguousarray(
        np.concatenate([W1[:, H:].T, b1[None, :]], axis=0)
    ) * 0.5  # [301, 100]

    # Packed W2 block weights: per m_g one [128, 224] window with nonzeros at
    # column 96 + p%32, value W2[4*m_g + p//32].  The lhsT for (m_g, i_g) is
    # the 128-column slice at offset 224*m_g + 96 - 32*i_g, which places the
    # nonzero at local column 32*i_g + p%32 — i.e. out row 32*i_g + p%32 —
    # summing the four m_l lanes into each i's score.
    w2blk = np.zeros((128, 5600), np.float32)
    P = np.arange(128)
    for m_g in range(25):
        w2blk[P, 224 * m_g + 96 + (P % 32)] = 0.5 * W2[0, 4 * m_g + P // 32]
    if not USE_F32R:
        w2blk = w2blk.astype(ml_dtypes.bfloat16)

    iota = np.ascontiguousarray(
        np.broadcast_to(np.arange(NP1, dtype=np.float32), (128, NP1))
    )
    b2s = np.full((128, 1), b2.reshape(-1)[0] + 0.5 * W2.sum(), np.float32)
    pat4 = np.zeros((100, 128), np.float32)
    pat4[np.arange(128) // 32, np.arange(128)] = 1.0
    pat4[32:36] = pat4[0:4]
    pat4[64:68] = pat4[0:4]
    pat4[96:100] = pat4[0:4]
    ones = np.ones((128, 1), np.float32)

    in_maps = []
    for k in range(NCORES):
        sl = slice(k * NLOC, (k + 1) * NLOC)
        in_maps.append(
            {
                "sentT": np.ascontiguousarray(sentT_all[:, sl]),
                "w1aTr": w1aTr,
                "w1bTx": w1bTx,
                "pmTx": pmTx,
                "w2blk": w2blk,
                "iota": iota,
                "tcol": target_scores[sl].astype(np.float32).reshape(NLOC, 1),
                "b2s": b2s,
                "ones128": ones,
                "pat4": pat4,
            }
        )
    return in_maps


def assemble_outputs(results):
    """results: list of 8 dicts with out_sm [128,1025] and out_loss [1,1]."""
    score_matrix = np.concatenate([r["out_sm"] for r in results], axis=0)
    loss_sum = sum(float(r["out_loss"][0, 0]) for r in results)
    loss = np.float32(loss_sum / (N * NP1))
    return loss, score_matrix


def run(inputs, **kwargs):
    from concourse.bass_utils import run_bass_kernel_spmd

    nc = get_program()
    in_maps = prep_inputs(
        inputs["sentence"],
        inputs["target_scores"],
        inputs["W1"],
        inputs["b1"],
        inputs["W2"],
        inputs["b2"],
    )
    res = run_bass_kernel_spmd(nc, in_maps, core_ids=list(range(NCORES)), **kwargs)
    return assemble_outputs(res.results), res


def kernel(**inputs):
    out, _ = run(inputs)
    return out
